# revision 1
# baseline (speedup 1.0000x reference)
"""Trainium2 Bass kernel for nn_Mixture_Loss_74053826118054.

Strategy (pure data parallel: batch axis B=256 sharded over 8 cores):
  Every term of the loss depends only on 5 per-(s,b)-row reductions over D:
    ll = sum_d l^2,  tt = sum_d t^2,  lt = sum_d l*t,
    ln = sum_d l[s]*l[s+1],  tn = sum_d t[s]*t[s+1]
  The tiny O(S*B) finish (cos, deltas, rank-compaction, delta-of-delta)
  runs on host in float64, reproducing the reference semantics exactly.

Mask compaction: every loss term sums only over valid (unmasked) rows and
over pairs of consecutive valid rows, and the mask is known on the host
before launch. Each core therefore receives only its VALID rows, densely
packed in original order (plus zero padding to a whole number of 16-row
windows). ll/tt/lt of valid rows are exact; consecutive-pair products of
the compacted stream cover every valid pair (r, r+1) (adjacent rows stay
adjacent), and false pairs across gaps are simply never read by the host
finish. With a ~50% random mask this roughly halves both DMA traffic and
DVE work. The program is built per window-count g (g = 16 == no masking
degrades to the dense kernel).

Engine assignment (from HW traces of earlier versions):
  - DVE fused scalar_tensor_tensor runs ~1221 ns/[128,1024] only while
    GpSimd is idle and at most 19 DMA jobs (all on the Sync ring) exist;
    GpSimd compute (2.8x), GpSimd-ring DMA or >19 jobs (1.2x), and PE
    streaming (1.2x) all slow DVE via SBUF/ring contention.
  - ACT: ll, tt squares with fused accumulate (contention-immune).
  - DVE: ln, tn, lt as fused stt product+accumulate; lt_j needs only
    chunk j so it runs one chunk ahead of ln_j/tn_j (fills the DMA ramp).
  - Chunks 0/1 are fetched as d-halves so DVE starts ~6 us earlier;
    the overlap chunk is issued mid-stream; row-interleaved DRAM layout
    gives one contiguous 8 KB read per partition-line; results ship as
    two output DMAs (all but the last chunk's columns early).
"""

import numpy as np

from contextlib import ExitStack

import concourse.bass as bass
import concourse.mybir as mybir
from concourse.bass_utils import run_bass_kernel_spmd

F32 = mybir.dt.float32
AF = mybir.ActivationFunctionType
ALU = mybir.AluOpType

N_CORES = 8
S, B, D = 64, 256, 1024
B_SHARD = B // N_CORES          # 32 batches per core
RAW_ROWS = B_SHARD * S          # 2048 rows per core before compaction
P = 128                         # partitions per tile
QUANTS = ("ll", "tt", "lt", "ln", "tn")

_cached = {}


def _build_program(g):
    """g = rows-per-partition window (chunks 0..g-1 plus overlap chunk g)."""
    if g in _cached:
        return _cached[g]
    rows_pad = (P + 1) * g
    nc = bass.Bass()
    x_in = nc.dram_tensor("x", [rows_pad, 2, D], F32, kind="ExternalInput")
    res_out = nc.dram_tensor("res", [P, 5 * g + 9], F32,
                             kind="ExternalOutput")
    x_v = x_in.rearrange("(w g) v d -> w g v d", g=g)

    with ExitStack() as stack:
        ec = stack.enter_context
        csem = [ec(nc.semaphore(f"c{j}")) for j in range(g + 1)]
        xbig = ec(nc.sbuf_tensor([P, (g + 1) * 2 * D], F32))
        dummies = ec(nc.sbuf_tensor([P, 8], F32))
        res = ec(nc.sbuf_tensor([P, 5 * g + 9], F32))
        ha0_sem = ec(nc.semaphore("ha0_sem"))
        ha1_sem = ec(nc.semaphore("ha1_sem"))
        part_sem = ec(nc.semaphore("part_sem"))
        done_sem = ec(nc.semaphore("done_sem"))
        out_sem = ec(nc.semaphore("out_sem"))
        block = ec(nc.Block())
        qidx = {q: i for i, q in enumerate(QUANTS)}
        xc = xbig.ap().rearrange("p (c v d) -> p c v d", v=2, d=D)

        def chunk(j, half, dslc=slice(None)):
            return xc[:, j, half, dslc]

        def rcol(q, j):
            k = 5 * j + qidx[q]
            return res.ap()[:, k:k + 1]

        def scol(k):
            return res.ap()[:, 5 * g + k:5 * g + k + 1]

        def bcast(k, n=D):
            return dummies.ap()[:, k:k + 1].broadcast_to((P, n))

        def semof(j):
            # chunks 13..g-1 arrive as one DMA job on csem[13] (keeps the
            # total job count <= 19, which HW traces showed is required
            # for full-rate DVE)
            return csem[j] if (j < 13 or j == g) else csem[13]

        HA = slice(0, D // 2)
        HB = slice(D // 2, D)
        jpart = g - 2           # both engines past this chunk -> ship out-a
        acols = 5 * (g - 1)     # out-a column count

        @block.sync
        def _(sync):
            sync.dma_start(out=xc[:, 0, :, HA],
                           in_=x_v[0:P, 0, :, HA]).then_inc(ha0_sem, 16)
            sync.dma_start(out=xc[:, 1, :, HA],
                           in_=x_v[0:P, 1, :, HA]).then_inc(ha1_sem, 16)
            sync.dma_start(out=xc[:, 0, :, HB],
                           in_=x_v[0:P, 0, :, HB]).then_inc(csem[0], 16)
            sync.dma_start(out=xc[:, 1, :, HB],
                           in_=x_v[0:P, 1, :, HB]).then_inc(csem[1], 16)
            for j in range(2, min(5, g)):
                sync.dma_start(out=xc[:, j, :, :],
                               in_=x_v[0:P, j, :, :]).then_inc(csem[j], 16)
            sync.dma_start(out=xc[:, g, :, :],
                           in_=x_v[1:P + 1, 0, :, :]).then_inc(csem[g], 16)
            for j in range(5, min(13, g)):
                sync.dma_start(out=xc[:, j, :, :],
                               in_=x_v[0:P, j, :, :]).then_inc(csem[j], 16)
            if g > 13:
                sync.dma_start(out=xc[:, 13:g, :, :],
                               in_=x_v[0:P, 13:g, :, :]).then_inc(
                    csem[13], 16)
            sync.wait_ge(part_sem, 2)
            sync.dma_start(out=res_out[:, 0:acols],
                           in_=res.ap()[:, 0:acols]).then_inc(out_sem, 16)
            sync.wait_ge(done_sem, 2)
            sync.dma_start(out=res_out[:, acols:5 * g + 9],
                           in_=res.ap()[:, acols:5 * g + 9]).then_inc(
                out_sem, 16)
            sync.wait_ge(out_sem, 32)

        @block.scalar
        def _(scalar):
            scalar.wait_ge(ha0_sem, 16)
            scalar.activation(bcast(0, D // 2), chunk(0, 0, HA), AF.Square,
                              accum_out=rcol("ll", 0))
            scalar.activation(bcast(1, D // 2), chunk(0, 1, HA), AF.Square,
                              accum_out=rcol("tt", 0))
            scalar.wait_ge(csem[0], 16)
            scalar.activation(bcast(0, D // 2), chunk(0, 0, HB), AF.Square,
                              accum_out=scol(qidx["ll"]))
            scalar.activation(bcast(1, D // 2), chunk(0, 1, HB), AF.Square,
                              accum_out=scol(qidx["tt"]))
            scalar.wait_ge(ha1_sem, 16)
            for j in range(1, g):
                if j < 14:
                    scalar.wait_ge(semof(j), 16)
                scalar.activation(bcast(0), chunk(j, 0), AF.Square,
                                  accum_out=rcol("ll", j))
                ins = scalar.activation(bcast(1), chunk(j, 1), AF.Square,
                                        accum_out=rcol("tt", j))
                if j == jpart:
                    ins.then_inc(part_sem, 1)
            ins.then_inc(done_sem, 1)

        @block.vector
        def _(vector):
            vector.wait_ge(ha0_sem, 16)
            vector.scalar_tensor_tensor(
                out=bcast(2, D // 2), in0=chunk(0, 0, HA), scalar=0.0,
                in1=chunk(0, 1, HA), op0=ALU.bypass, op1=ALU.mult,
                accum_out=rcol("lt", 0))
            vector.wait_ge(ha1_sem, 16)
            vector.scalar_tensor_tensor(
                out=bcast(3, D // 2), in0=chunk(0, 0, HA), scalar=0.0,
                in1=chunk(1, 0, HA), op0=ALU.bypass, op1=ALU.mult,
                accum_out=rcol("ln", 0))
            vector.scalar_tensor_tensor(
                out=bcast(4, D // 2), in0=chunk(0, 1, HA), scalar=0.0,
                in1=chunk(1, 1, HA), op0=ALU.bypass, op1=ALU.mult,
                accum_out=rcol("tn", 0))
            vector.wait_ge(csem[0], 16)
            vector.scalar_tensor_tensor(
                out=bcast(2, D // 2), in0=chunk(0, 0, HB), scalar=0.0,
                in1=chunk(0, 1, HB), op0=ALU.bypass, op1=ALU.mult,
                accum_out=scol(qidx["lt"]))
            vector.wait_ge(csem[1], 16)
            vector.scalar_tensor_tensor(
                out=bcast(3, D // 2), in0=chunk(0, 0, HB), scalar=0.0,
                in1=chunk(1, 0, HB), op0=ALU.bypass, op1=ALU.mult,
                accum_out=scol(qidx["ln"]))
            vector.scalar_tensor_tensor(
                out=bcast(4, D // 2), in0=chunk(0, 1, HB), scalar=0.0,
                in1=chunk(1, 1, HB), op0=ALU.bypass, op1=ALU.mult,
                accum_out=scol(qidx["tn"]))
            # lt_j needs only chunk j: run it one chunk ahead of ln/tn
            vector.scalar_tensor_tensor(
                out=bcast(2), in0=chunk(1, 0), scalar=0.0,
                in1=chunk(1, 1), op0=ALU.bypass, op1=ALU.mult,
                accum_out=rcol("lt", 1))
            for j in range(1, g):
                vector.wait_ge(semof(j + 1), 16)
                if j < g - 1:
                    vector.scalar_tensor_tensor(
                        out=bcast(2), in0=chunk(j + 1, 0), scalar=0.0,
                        in1=chunk(j + 1, 1), op0=ALU.bypass, op1=ALU.mult,
                        accum_out=rcol("lt", j + 1))
                vector.scalar_tensor_tensor(
                    out=bcast(3), in0=chunk(j, 0), scalar=0.0,
                    in1=chunk(j + 1, 0), op0=ALU.bypass, op1=ALU.mult,
                    accum_out=rcol("ln", j))
                ins = vector.scalar_tensor_tensor(
                    out=bcast(4), in0=chunk(j, 1), scalar=0.0,
                    in1=chunk(j + 1, 1), op0=ALU.bypass, op1=ALU.mult,
                    accum_out=rcol("tn", j))
                if j == jpart:
                    ins.then_inc(part_sem, 1)
            ins.then_inc(done_sem, 1)

    _cached[g] = nc
    return nc


def _run_device(logits, tgt_out, valid, trace=False):
    """Returns dict q -> (B, S) float32 row-dot arrays (zeros at unused
    positions), plus kernel results."""
    lbg = np.ascontiguousarray(np.swapaxes(logits, 0, 1)).reshape(B * S, D)
    tbg = np.ascontiguousarray(np.swapaxes(tgt_out, 0, 1)).reshape(B * S, D)
    nrows = B * S
    vflat = valid.reshape(-1)
    vp_all = np.flatnonzero(vflat)
    tot = len(vp_all)
    # balanced split of the global valid-row stream; cuts land where the
    # two rows either side are NOT an adjacent valid pair, so no valid
    # pair straddles cores
    cuts = [0]
    for c in range(1, N_CORES):
        t = (tot * c) // N_CORES
        while 0 < t < tot and vp_all[t] == vp_all[t - 1] + 1:
            t += 1
        cuts.append(min(t, tot))
    cuts.append(tot)
    vps = [vp_all[cuts[c]:cuts[c + 1]] for c in range(N_CORES)]
    kmax = max((len(vp) for vp in vps), default=1)
    g = min(16, max(4, -(-kmax // P)))
    rows_c = P * g
    rows_pad = (P + 1) * g
    nc = _build_program(g)
    in_maps = []
    for c in range(N_CORES):
        vp = vps[c]
        x = np.zeros((rows_pad, 2, D), np.float32)
        x[:len(vp), 0] = lbg[vp]
        x[:len(vp), 1] = tbg[vp]
        in_maps.append({"x": x})
    kres = run_bass_kernel_spmd(nc, in_maps, list(range(N_CORES)), trace=trace)
    pos = np.cumsum(vflat) - 1                 # orig row -> compacted idx
    rs = np.flatnonzero(vflat[:-1] & vflat[1:])
    full = {}
    for i, q in enumerate(QUANTS):
        cat = np.zeros(tot + 1, np.float64)
        for c in range(N_CORES):
            r = kres.results[c]["res"]
            arr = r[:, :5 * g].reshape(P, g, 5)[:, :, i].copy()
            arr[:, 0] += r[:, 5 * g + i]       # chunk 0's B-half
            vals = arr.reshape(rows_c)         # compacted row r = g*p + j
            cat[cuts[c]:cuts[c + 1]] = vals[:len(vps[c])]
        fullr = np.zeros(nrows, np.float64)
        if q in ("ll", "tt", "lt"):
            fullr[vp_all] = cat[:tot]
        else:
            # pair (r, r+1), both valid: adjacent compacted positions in
            # one core's stream (cuts avoid valid pairs)
            fullr[rs] = cat[pos[rs]]
        full[q] = fullr.reshape(B, S)
    return full, kres


def _finish_host(rows, mask):
    """Host-side float64 finish: reproduce reference semantics exactly."""
    ll = rows["ll"].astype(np.float64)
    tt = rows["tt"].astype(np.float64)
    lt = rows["lt"].astype(np.float64)
    ln = rows["ln"].astype(np.float64)
    tn = rows["tn"].astype(np.float64)

    valid = ~mask                     # (B, S)
    n_valid = float(valid.sum())

    mse = ((ll - 2.0 * lt + tt) * valid).sum() / (n_valid * D)

    na = np.maximum(np.sqrt(ll), 1e-8)
    nb = np.maximum(np.sqrt(tt), 1e-8)
    c = np.where(valid, lt / (na * nb), 0.0)
    loss_cos = ((1.0 - c) * valid).sum() / n_valid

    nl = np.maximum(np.sqrt(ll), 1e-6)
    nt = np.maximum(np.sqrt(tt), 1e-6)
    d_l = ln[:, :S - 1] / (nl[:, :-1] * nl[:, 1:])
    d_t = tn[:, :S - 1] / (nt[:, :-1] * nt[:, 1:])
    pair_valid = valid[:, :-1] & valid[:, 1:]
    cnt = int(pair_valid.sum())
    loss_delta = (np.square(d_l - d_t) * pair_valid).sum() / max(cnt, 1)

    L = B * (S - 1)
    pvf = pair_valid.reshape(-1)

    def dd(d_flat):
        dense = np.zeros(L, np.float64)
        dense[:cnt] = d_flat[pvf]
        prev = dense[:-1]
        den = np.where(prev != 0, prev, 1e-6)
        return (dense[1:] - prev) / den

    dd_l = dd(d_l.reshape(-1))
    dd_t = dd(d_t.reshape(-1))
    dd_valid = np.arange(L - 1) < (cnt - 1)
    n_dd = float(max(cnt - 1, 1))
    loss_dd = (np.square(dd_l - dd_t) * dd_valid).sum() / n_dd / 100.0

    return mse + loss_cos + loss_delta + loss_dd


def kernel(logits, tgt_out, tgt_padding_mask, _trace=False):
    logits = np.asarray(logits, dtype=np.float32)
    tgt_out = np.asarray(tgt_out, dtype=np.float32)
    mask = np.asarray(tgt_padding_mask).astype(bool)
    rows, kres = _run_device(logits, tgt_out, ~mask, trace=_trace)
    total = _finish_host(rows, mask)
    out = np.array(total, dtype=np.float32)
    if _trace:
        return out, kres
    return out



# revision 8
# speedup vs baseline: 1.0099x; 1.0099x over previous
"""Trainium2 Bass kernel for nn_Mixture_Loss_74053826118054.

Strategy (pure data parallel: batch axis B=256 sharded over 8 cores):
  Every term of the loss depends only on 5 per-(s,b)-row reductions over D:
    ll = sum_d l^2,  tt = sum_d t^2,  lt = sum_d l*t,
    ln = sum_d l[s]*l[s+1],  tn = sum_d t[s]*t[s+1]
  The tiny O(S*B) finish (cos, deltas, rank-compaction, delta-of-delta)
  runs on host in float64, reproducing the reference semantics exactly.

Mask compaction: each core receives only its VALID rows, densely packed in
original order. Row r maps to (partition p, chunk j) with r = g*p + j, so
consecutive rows stay adjacent within a partition's g-row window; the
window-boundary pairs (r = g*p + g-1) are computed on the HOST (127 pairs
per core, microseconds in numpy) so no overlap chunk is ever DMA'd.
DMA is trimmed to parts = ceil(kmax/g) partitions so only real rows move.

Engine assignment (v3, from HW traces):
  - ACT: ll, tt squares (1.04 us/[128,1024]) + Copy-accumulate reductions
    of the Pool products.
  - DVE: lt (one chunk ahead), ln, and tn for non-pool chunks via fused
    scalar_tensor_tensor (1.14 us/op, no fast mode exists for f32).
  - Pool/GpSimd: tn elementwise products for ~half the chunks (software
    tensor_tensor mult, ~2 us/op) into a 3-slot ring; ACT reduces them
    two iterations later so neither engine stalls.
  This balances DVE ~21 us / ACT ~21 us per core against the ~20 us DMA
  roofline (16 rings x ~26.7 GB/s) for the ~8.4 MB/core of f32 rows.
  f32 is mandatory: loss_dd amplifies tiny cosine-delta errors ~1000x
  (fp16/bf16 inputs fail by 25x/126x).
"""

import numpy as np

from contextlib import ExitStack

import concourse.bass as bass
import concourse.mybir as mybir
from concourse.bass_utils import run_bass_kernel_spmd

F32 = mybir.dt.float32
AF = mybir.ActivationFunctionType
ALU = mybir.AluOpType

N_CORES = 8
S, B, D = 64, 256, 1024
P = 128                         # max partitions per tile
QUANTS = ("ll", "tt", "lt", "ln", "tn")

_cached = {}


def _pool_chunks(g):
    """Chunks whose tn product runs on Pool (from 1..g-3, evenly spread)."""
    k = int(round((1.34 * g - 2.28) / 2.18))
    k = max(0, min(k, max(g - 3, 0)))
    cand = list(range(1, g - 2))
    if k <= 0 or not cand:
        return []
    if k >= len(cand):
        return cand
    if k == 1:
        return [cand[0]]
    idxs = sorted({round(i * (len(cand) - 1) / (k - 1)) for i in range(k)})
    return [cand[i] for i in idxs]


def _build_program(g, parts):
    """g = rows per partition window; parts = partitions carrying rows."""
    key = (g, parts)
    if key in _cached:
        return _cached[key]
    rows_c = parts * g
    pool_set = _pool_chunks(g)
    pool_ord = {j: i for i, j in enumerate(pool_set)}
    nc = bass.Bass()
    ncols = 5 * g + 5
    x_in = nc.dram_tensor("x", [rows_c, 2, D], F32, kind="ExternalInput")
    res_out = nc.dram_tensor("res", [parts, ncols], F32,
                             kind="ExternalOutput")
    x_v = x_in.rearrange("(w g) v d -> w g v d", g=g)

    with ExitStack() as stack:
        ec = stack.enter_context
        n_csem = min(g, 14)
        csem = [ec(nc.semaphore(f"c{j}")) for j in range(n_csem)]
        xbig = ec(nc.sbuf_tensor([P, g * 2 * D], F32))
        prod = ec(nc.sbuf_tensor([P, 3 * D], F32))
        dummies = ec(nc.sbuf_tensor([P, 8], F32))
        res = ec(nc.sbuf_tensor([P, ncols], F32))
        ha0_sem = ec(nc.semaphore("ha0_sem"))
        ha1_sem = ec(nc.semaphore("ha1_sem"))
        pmul_sem = ec(nc.semaphore("pmul_sem"))
        pcons_sem = ec(nc.semaphore("pcons_sem"))
        part_sem = ec(nc.semaphore("part_sem"))
        done_sem = ec(nc.semaphore("done_sem"))
        out_sem = ec(nc.semaphore("out_sem"))
        block = ec(nc.Block())
        qidx = {q: i for i, q in enumerate(QUANTS)}
        xc = xbig.ap().rearrange("p (c v d) -> p c v d", v=2, d=D)
        pslot = prod.ap().rearrange("p (s d) -> p s d", d=D)

        def chunk(j, half, dslc=slice(None)):
            return xc[0:parts, j, half, dslc]

        def rcol(q, j):
            k = 5 * j + qidx[q]
            return res.ap()[0:parts, k:k + 1]

        def scol(k):
            return res.ap()[0:parts, 5 * g + k:5 * g + k + 1]

        def bcast(k, n=D):
            return dummies.ap()[0:parts, k:k + 1].broadcast_to((parts, n))

        def semof(j):
            # chunks 13..g-1 arrive as one DMA job on csem[13] (keeps the
            # total job count <= 19, required for full-rate DVE)
            return csem[j] if j < 13 else csem[13]

        HA = slice(0, D // 2)
        HB = slice(D // 2, D)
        acols = 5 * (g - 1)     # out-a column count (chunks 0..g-2)

        @block.sync
        def _(sync):
            sync.dma_start(out=xc[0:parts, 0, :, HA],
                           in_=x_v[0:parts, 0, :, HA]).then_inc(ha0_sem, 16)
            sync.dma_start(out=xc[0:parts, 1, :, HA],
                           in_=x_v[0:parts, 1, :, HA]).then_inc(ha1_sem, 16)
            sync.dma_start(out=xc[0:parts, 0, :, HB],
                           in_=x_v[0:parts, 0, :, HB]).then_inc(csem[0], 16)
            sync.dma_start(out=xc[0:parts, 1, :, HB],
                           in_=x_v[0:parts, 1, :, HB]).then_inc(csem[1], 16)
            for j in range(2, min(13, g)):
                sync.dma_start(out=xc[0:parts, j, :, :],
                               in_=x_v[0:parts, j, :, :]).then_inc(csem[j], 16)
            if g > 13:
                sync.dma_start(out=xc[0:parts, 13:g, :, :],
                               in_=x_v[0:parts, 13:g, :, :]).then_inc(
                    csem[13], 16)
            sync.wait_ge(part_sem, 2)
            sync.dma_start(out=res_out[:, 0:acols],
                           in_=res.ap()[0:parts, 0:acols]).then_inc(out_sem, 16)
            sync.wait_ge(done_sem, 2)
            sync.dma_start(out=res_out[:, acols:ncols],
                           in_=res.ap()[0:parts, acols:ncols]).then_inc(
                out_sem, 16)
            sync.wait_ge(out_sem, 32)

        # ACT part carrier: last op writing a chunk<=g-2 column
        act_part_reduce = (g - 3) if (g - 3) in pool_set else None

        @block.scalar
        def _(scalar):
            scalar.wait_ge(ha0_sem, 16)
            scalar.activation(bcast(0, D // 2), chunk(0, 0, HA), AF.Square,
                              accum_out=rcol("ll", 0))
            scalar.activation(bcast(1, D // 2), chunk(0, 1, HA), AF.Square,
                              accum_out=rcol("tt", 0))
            scalar.wait_ge(csem[0], 16)
            scalar.activation(bcast(0, D // 2), chunk(0, 0, HB), AF.Square,
                              accum_out=scol(qidx["ll"]))
            scalar.activation(bcast(1, D // 2), chunk(0, 1, HB), AF.Square,
                              accum_out=scol(qidx["tt"]))
            scalar.wait_ge(ha1_sem, 16)
            for j in range(1, g):
                if j < 14:
                    scalar.wait_ge(semof(j), 16)
                jr = j - 2          # pool reduce lags its mult by 2 chunks
                if jr in pool_ord:
                    o = pool_ord[jr]
                    scalar.wait_ge(pmul_sem, o + 1)
                    scalar.activation(
                        bcast(5), pslot[0:parts, o % 3, :], AF.Copy,
                        accum_out=rcol("tn", jr)).then_inc(pcons_sem, 1)
                ins = scalar.activation(bcast(0), chunk(j, 0), AF.Square,
                                        accum_out=rcol("ll", j))
                if j == g - 1 and act_part_reduce is not None:
                    # runs after the last pool reduce (chunk g-3's column)
                    ins.then_inc(part_sem, 1)
                ins = scalar.activation(bcast(1), chunk(j, 1), AF.Square,
                                        accum_out=rcol("tt", j))
                if j == g - 2 and act_part_reduce is None:
                    ins.then_inc(part_sem, 1)
            ins.then_inc(done_sem, 1)

        @block.vector
        def _(vector):
            vector.wait_ge(ha0_sem, 16)
            vector.scalar_tensor_tensor(
                out=bcast(2, D // 2), in0=chunk(0, 0, HA), scalar=0.0,
                in1=chunk(0, 1, HA), op0=ALU.bypass, op1=ALU.mult,
                accum_out=rcol("lt", 0))
            vector.wait_ge(ha1_sem, 16)
            vector.scalar_tensor_tensor(
                out=bcast(3, D // 2), in0=chunk(0, 0, HA), scalar=0.0,
                in1=chunk(1, 0, HA), op0=ALU.bypass, op1=ALU.mult,
                accum_out=rcol("ln", 0))
            vector.scalar_tensor_tensor(
                out=bcast(4, D // 2), in0=chunk(0, 1, HA), scalar=0.0,
                in1=chunk(1, 1, HA), op0=ALU.bypass, op1=ALU.mult,
                accum_out=rcol("tn", 0))
            vector.wait_ge(csem[0], 16)
            vector.scalar_tensor_tensor(
                out=bcast(2, D // 2), in0=chunk(0, 0, HB), scalar=0.0,
                in1=chunk(0, 1, HB), op0=ALU.bypass, op1=ALU.mult,
                accum_out=scol(qidx["lt"]))
            vector.wait_ge(csem[1], 16)
            vector.scalar_tensor_tensor(
                out=bcast(3, D // 2), in0=chunk(0, 0, HB), scalar=0.0,
                in1=chunk(1, 0, HB), op0=ALU.bypass, op1=ALU.mult,
                accum_out=scol(qidx["ln"]))
            vector.scalar_tensor_tensor(
                out=bcast(4, D // 2), in0=chunk(0, 1, HB), scalar=0.0,
                in1=chunk(1, 1, HB), op0=ALU.bypass, op1=ALU.mult,
                accum_out=scol(qidx["tn"]))
            # lt_j needs only chunk j: run it one chunk ahead of ln/tn
            vector.scalar_tensor_tensor(
                out=bcast(2), in0=chunk(1, 0), scalar=0.0,
                in1=chunk(1, 1), op0=ALU.bypass, op1=ALU.mult,
                accum_out=rcol("lt", 1))
            for j in range(1, g - 1):
                vector.wait_ge(semof(j + 1), 16)
                if j < g - 2:
                    vector.scalar_tensor_tensor(
                        out=bcast(2), in0=chunk(j + 1, 0), scalar=0.0,
                        in1=chunk(j + 1, 1), op0=ALU.bypass, op1=ALU.mult,
                        accum_out=rcol("lt", j + 1))
                ins = vector.scalar_tensor_tensor(
                    out=bcast(3), in0=chunk(j, 0), scalar=0.0,
                    in1=chunk(j + 1, 0), op0=ALU.bypass, op1=ALU.mult,
                    accum_out=rcol("ln", j))
                if j not in pool_ord:
                    ins = vector.scalar_tensor_tensor(
                        out=bcast(4), in0=chunk(j, 1), scalar=0.0,
                        in1=chunk(j + 1, 1), op0=ALU.bypass, op1=ALU.mult,
                        accum_out=rcol("tn", j))
                if j == g - 2:
                    ins.then_inc(part_sem, 1)
            # last chunk's lt, deferred so part_sem fires as early as possible
            vector.scalar_tensor_tensor(
                out=bcast(2), in0=chunk(g - 1, 0), scalar=0.0,
                in1=chunk(g - 1, 1), op0=ALU.bypass, op1=ALU.mult,
                accum_out=rcol("lt", g - 1)).then_inc(done_sem, 1)

        @block.gpsimd
        def _(gpsimd):
            # the j = g-1 ln/tn columns are host-filled; zero them so the
            # out-b DMA never reads uninitialized SBUF
            gpsimd.memset(res.ap()[0:parts, acols + 3:acols + 5], 0)
            for o, j in enumerate(pool_set):
                gpsimd.wait_ge(semof(j + 1), 16)
                if o >= 3:
                    gpsimd.wait_ge(pcons_sem, o - 2)
                gpsimd.tensor_tensor(
                    out=pslot[0:parts, o % 3, :], in0=chunk(j, 1),
                    in1=chunk(j + 1, 1), op=ALU.mult).then_inc(pmul_sem, 1)

    _cached[key] = nc
    return nc


def _run_device(logits, tgt_out, valid, trace=False):
    """Returns dict q -> (B, S) float32 row-dot arrays (zeros at unused
    positions), plus kernel results."""
    lbg = np.ascontiguousarray(np.swapaxes(logits, 0, 1)).reshape(B * S, D)
    tbg = np.ascontiguousarray(np.swapaxes(tgt_out, 0, 1)).reshape(B * S, D)
    nrows = B * S
    vflat = valid.reshape(-1)
    vp_all = np.flatnonzero(vflat)
    tot = len(vp_all)
    # balanced split of the global valid-row stream; cuts land where the
    # two rows either side are NOT an adjacent valid pair, so no valid
    # pair straddles cores
    cuts = [0]
    for c in range(1, N_CORES):
        t = (tot * c) // N_CORES
        while 0 < t < tot and vp_all[t] == vp_all[t - 1] + 1:
            t += 1
        cuts.append(min(t, tot))
    cuts.append(tot)
    vps = [vp_all[cuts[c]:cuts[c + 1]] for c in range(N_CORES)]
    kmax = max((len(vp) for vp in vps), default=1)
    g = min(16, max(4, -(-kmax // P)))
    parts = min(P, -(-kmax // g))
    rows_c = parts * g
    nc = _build_program(g, parts)
    in_maps = []
    for c in range(N_CORES):
        vp = vps[c]
        x = np.zeros((rows_c, 2, D), np.float32)
        x[:len(vp), 0] = lbg[vp]
        x[:len(vp), 1] = tbg[vp]
        in_maps.append({"x": x})
    kres = run_bass_kernel_spmd(nc, in_maps, list(range(N_CORES)), trace=trace)
    # host-side boundary pairs: compacted rows (g*p+g-1, g*p+g) share no
    # chunk window on-device; their ln/tn come from a tiny host einsum
    hb = {}
    for c in range(N_CORES):
        vp = vps[c]
        r = np.arange(g - 1, rows_c - 1, g)
        r = r[r + 1 < len(vp)]
        a, b = vp[r], vp[r + 1]
        hb[c] = (r,
                 np.einsum('ij,ij->i', lbg[a], lbg[b], dtype=np.float64),
                 np.einsum('ij,ij->i', tbg[a], tbg[b], dtype=np.float64))
    pos = np.cumsum(vflat) - 1                 # orig row -> compacted idx
    rs = np.flatnonzero(vflat[:-1] & vflat[1:])
    full = {}
    for i, q in enumerate(QUANTS):
        cat = np.zeros(tot + 1, np.float64)
        for c in range(N_CORES):
            r = kres.results[c]["res"]
            arr = r[:, :5 * g].reshape(parts, g, 5)[:, :, i].copy()
            arr[:, 0] += r[:, 5 * g + i]       # chunk 0's B-half
            vals = arr.reshape(rows_c).astype(np.float64)
            if q in ("ln", "tn"):
                ridx, lnb, tnb = hb[c]
                vals[ridx] = lnb if q == "ln" else tnb
            cat[cuts[c]:cuts[c + 1]] = vals[:len(vps[c])]
        fullr = np.zeros(nrows, np.float64)
        if q in ("ll", "tt", "lt"):
            fullr[vp_all] = cat[:tot]
        else:
            # pair (r, r+1), both valid: adjacent compacted positions in
            # one core's stream (cuts avoid valid pairs)
            fullr[rs] = cat[pos[rs]]
        full[q] = fullr.reshape(B, S)
    return full, kres


def _finish_host(rows, mask):
    """Host-side float64 finish: reproduce reference semantics exactly."""
    ll = rows["ll"].astype(np.float64)
    tt = rows["tt"].astype(np.float64)
    lt = rows["lt"].astype(np.float64)
    ln = rows["ln"].astype(np.float64)
    tn = rows["tn"].astype(np.float64)

    valid = ~mask                     # (B, S)
    n_valid = float(valid.sum())

    mse = ((ll - 2.0 * lt + tt) * valid).sum() / (n_valid * D)

    na = np.maximum(np.sqrt(ll), 1e-8)
    nb = np.maximum(np.sqrt(tt), 1e-8)
    c = np.where(valid, lt / (na * nb), 0.0)
    loss_cos = ((1.0 - c) * valid).sum() / n_valid

    nl = np.maximum(np.sqrt(ll), 1e-6)
    nt = np.maximum(np.sqrt(tt), 1e-6)
    d_l = ln[:, :S - 1] / (nl[:, :-1] * nl[:, 1:])
    d_t = tn[:, :S - 1] / (nt[:, :-1] * nt[:, 1:])
    pair_valid = valid[:, :-1] & valid[:, 1:]
    cnt = int(pair_valid.sum())
    loss_delta = (np.square(d_l - d_t) * pair_valid).sum() / max(cnt, 1)

    L = B * (S - 1)
    pvf = pair_valid.reshape(-1)

    def dd(d_flat):
        dense = np.zeros(L, np.float64)
        dense[:cnt] = d_flat[pvf]
        prev = dense[:-1]
        den = np.where(prev != 0, prev, 1e-6)
        return (dense[1:] - prev) / den

    dd_l = dd(d_l.reshape(-1))
    dd_t = dd(d_t.reshape(-1))
    dd_valid = np.arange(L - 1) < (cnt - 1)
    n_dd = float(max(cnt - 1, 1))
    loss_dd = (np.square(dd_l - dd_t) * dd_valid).sum() / n_dd / 100.0

    return mse + loss_cos + loss_delta + loss_dd


def kernel(logits, tgt_out, tgt_padding_mask, _trace=False):
    logits = np.asarray(logits, dtype=np.float32)
    tgt_out = np.asarray(tgt_out, dtype=np.float32)
    mask = np.asarray(tgt_padding_mask).astype(bool)
    rows, kres = _run_device(logits, tgt_out, ~mask, trace=_trace)
    total = _finish_host(rows, mask)
    out = np.array(total, dtype=np.float32)
    if _trace:
        return out, kres
    return out


# revision 9
# speedup vs baseline: 1.1093x; 1.0984x over previous
"""Trainium2 Bass kernel for nn_Mixture_Loss_74053826118054.

Strategy (pure data parallel: batch axis B=256 sharded over 8 cores):
  Every term of the loss depends only on 5 per-(s,b)-row reductions over D:
    ll = sum_d l^2,  tt = sum_d t^2,  lt = sum_d l*t,
    ln = sum_d l[s]*l[s+1],  tn = sum_d t[s]*t[s+1]
  The tiny O(S*B) finish (cos, deltas, rank-compaction, delta-of-delta)
  runs on host in float64, reproducing the reference semantics exactly.

Mask compaction: each core receives only its VALID rows, densely packed in
original order. Row r maps to (partition p, chunk j) with r = g*p + j, so
consecutive rows stay adjacent within a partition's g-row window; the
window-boundary pairs (r = g*p + g-1) are computed on the HOST (127 pairs
per core, microseconds in numpy) so no overlap chunk is ever DMA'd.
DMA is trimmed to parts = ceil(kmax/g) partitions so only real rows move.

Engine assignment (v3, from HW traces):
  - ACT: ll, tt squares (1.04 us/[128,1024]) + Copy-accumulate reductions
    of the Pool products.
  - DVE: lt (one chunk ahead), ln, and tn for non-pool chunks via fused
    scalar_tensor_tensor (1.14 us/op, no fast mode exists for f32).
  - Pool/GpSimd: tn elementwise products for ~half the chunks (software
    tensor_tensor mult, ~2 us/op) into a 3-slot ring; ACT reduces them
    two iterations later so neither engine stalls.
  This balances DVE ~21 us / ACT ~21 us per core against the ~20 us DMA
  roofline (16 rings x ~26.7 GB/s) for the ~8.4 MB/core of f32 rows.
  f32 is mandatory: loss_dd amplifies tiny cosine-delta errors ~1000x
  (fp16/bf16 inputs fail by 25x/126x).
"""

import numpy as np

from contextlib import ExitStack

import concourse.bass as bass
import concourse.mybir as mybir
from concourse.bass_utils import run_bass_kernel_spmd

F32 = mybir.dt.float32
AF = mybir.ActivationFunctionType
ALU = mybir.AluOpType

N_CORES = 8
S, B, D = 64, 256, 1024
P = 128                         # max partitions per tile
QUANTS = ("ll", "tt", "lt", "ln", "tn")

_cached = {}


def _pool_chunks(g):
    """Chunks whose tn product runs on Pool (from 1..g-3, evenly spread)."""
    k = int(round((1.34 * g - 2.28) / 2.18))
    k = max(0, min(k, max(g - 3, 0)))
    cand = list(range(1, g - 2))
    if k <= 0 or not cand:
        return []
    if k >= len(cand):
        return cand
    if k == 1:
        return [cand[0]]
    idxs = sorted({round(i * (len(cand) - 1) / (k - 1)) for i in range(k)})
    return [cand[i] for i in idxs]


def _build_program(g, parts):
    """g = rows per partition window; parts = partitions carrying rows."""
    key = (g, parts)
    if key in _cached:
        return _cached[key]
    rows_c = parts * g
    pool_set = []
    pool_ord = {}
    nc = bass.Bass()
    ncols = 5 * g + 5
    x_in = nc.dram_tensor("x", [rows_c, 2, D], F32, kind="ExternalInput")
    res_out = nc.dram_tensor("res", [parts, ncols], F32,
                             kind="ExternalOutput")
    x_v = x_in.rearrange("(w g) v d -> w g v d", g=g)

    with ExitStack() as stack:
        ec = stack.enter_context
        n_csem = min(g, 14)
        csem = [ec(nc.semaphore(f"c{j}")) for j in range(n_csem)]
        xbig = ec(nc.sbuf_tensor([P, g * 2 * D], F32))
        prod = ec(nc.sbuf_tensor([P, 3 * D], F32))
        dummies = ec(nc.sbuf_tensor([P, 8], F32))
        res = ec(nc.sbuf_tensor([P, ncols], F32))
        ha0_sem = ec(nc.semaphore("ha0_sem"))
        ha1_sem = ec(nc.semaphore("ha1_sem"))
        pmul_sem = ec(nc.semaphore("pmul_sem"))
        pcons_sem = ec(nc.semaphore("pcons_sem"))
        part_sem = ec(nc.semaphore("part_sem"))
        done_sem = ec(nc.semaphore("done_sem"))
        out_sem = ec(nc.semaphore("out_sem"))
        block = ec(nc.Block())
        qidx = {q: i for i, q in enumerate(QUANTS)}
        xc = xbig.ap().rearrange("p (c v d) -> p c v d", v=2, d=D)
        pslot = prod.ap().rearrange("p (s d) -> p s d", d=D)

        def chunk(j, half, dslc=slice(None)):
            return xc[0:parts, j, half, dslc]

        def rcol(q, j):
            k = 5 * j + qidx[q]
            return res.ap()[0:parts, k:k + 1]

        def scol(k):
            return res.ap()[0:parts, 5 * g + k:5 * g + k + 1]

        def bcast(k, n=D):
            return dummies.ap()[0:parts, k:k + 1].broadcast_to((parts, n))

        def semof(j):
            # chunks 13..g-1 arrive as one DMA job on csem[13] (keeps the
            # total job count <= 19, required for full-rate DVE)
            return csem[j] if j < 13 else csem[13]

        HA = slice(0, D // 2)
        HB = slice(D // 2, D)
        acols = 5 * (g - 1)     # out-a column count (chunks 0..g-2)

        @block.sync
        def _(sync):
            sync.dma_start(out=xc[0:parts, 0, :, HA],
                           in_=x_v[0:parts, 0, :, HA]).then_inc(ha0_sem, 16)
            sync.dma_start(out=xc[0:parts, 1, :, HA],
                           in_=x_v[0:parts, 1, :, HA]).then_inc(ha1_sem, 16)
            sync.dma_start(out=xc[0:parts, 0, :, HB],
                           in_=x_v[0:parts, 0, :, HB]).then_inc(csem[0], 16)
            sync.dma_start(out=xc[0:parts, 1, :, HB],
                           in_=x_v[0:parts, 1, :, HB]).then_inc(csem[1], 16)
            for j in range(2, min(13, g)):
                sync.dma_start(out=xc[0:parts, j, :, :],
                               in_=x_v[0:parts, j, :, :]).then_inc(csem[j], 16)
            if g > 13:
                sync.dma_start(out=xc[0:parts, 13:g, :, :],
                               in_=x_v[0:parts, 13:g, :, :]).then_inc(
                    csem[13], 16)
            sync.wait_ge(part_sem, 2)
            sync.dma_start(out=res_out[:, 0:acols],
                           in_=res.ap()[0:parts, 0:acols]).then_inc(out_sem, 16)
            sync.wait_ge(done_sem, 2)
            sync.dma_start(out=res_out[:, acols:ncols],
                           in_=res.ap()[0:parts, acols:ncols]).then_inc(
                out_sem, 16)
            sync.wait_ge(out_sem, 32)

        # ACT part carrier: last op writing a chunk<=g-2 column
        act_part_reduce = (g - 3) if (g - 3) in pool_set else None

        @block.scalar
        def _(scalar):
            scalar.wait_ge(ha0_sem, 16)
            scalar.activation(bcast(0, D // 2), chunk(0, 0, HA), AF.Square,
                              accum_out=rcol("ll", 0))
            scalar.activation(bcast(1, D // 2), chunk(0, 1, HA), AF.Square,
                              accum_out=rcol("tt", 0))
            scalar.wait_ge(csem[0], 16)
            scalar.activation(bcast(0, D // 2), chunk(0, 0, HB), AF.Square,
                              accum_out=scol(qidx["ll"]))
            scalar.activation(bcast(1, D // 2), chunk(0, 1, HB), AF.Square,
                              accum_out=scol(qidx["tt"]))
            scalar.wait_ge(ha1_sem, 16)
            for j in range(1, g):
                if j < 14:
                    scalar.wait_ge(semof(j), 16)
                jr = j - 2          # pool reduce lags its mult by 2 chunks
                if jr in pool_ord:
                    o = pool_ord[jr]
                    scalar.wait_ge(pmul_sem, o + 1)
                    scalar.activation(
                        bcast(5), pslot[0:parts, o % 3, :], AF.Copy,
                        accum_out=rcol("tn", jr)).then_inc(pcons_sem, 1)
                ins = scalar.activation(bcast(0), chunk(j, 0), AF.Square,
                                        accum_out=rcol("ll", j))
                if j == g - 1 and act_part_reduce is not None:
                    # runs after the last pool reduce (chunk g-3's column)
                    ins.then_inc(part_sem, 1)
                ins = scalar.activation(bcast(1), chunk(j, 1), AF.Square,
                                        accum_out=rcol("tt", j))
                if j == g - 2 and act_part_reduce is None:
                    ins.then_inc(part_sem, 1)
            ins.then_inc(done_sem, 1)

        @block.vector
        def _(vector):
            vector.wait_ge(ha0_sem, 16)
            vector.scalar_tensor_tensor(
                out=bcast(2, D // 2), in0=chunk(0, 0, HA), scalar=0.0,
                in1=chunk(0, 1, HA), op0=ALU.bypass, op1=ALU.mult,
                accum_out=rcol("lt", 0))
            vector.wait_ge(ha1_sem, 16)
            vector.scalar_tensor_tensor(
                out=bcast(3, D // 2), in0=chunk(0, 0, HA), scalar=0.0,
                in1=chunk(1, 0, HA), op0=ALU.bypass, op1=ALU.mult,
                accum_out=rcol("ln", 0))
            vector.scalar_tensor_tensor(
                out=bcast(4, D // 2), in0=chunk(0, 1, HA), scalar=0.0,
                in1=chunk(1, 1, HA), op0=ALU.bypass, op1=ALU.mult,
                accum_out=rcol("tn", 0))
            vector.wait_ge(csem[0], 16)
            vector.scalar_tensor_tensor(
                out=bcast(2, D // 2), in0=chunk(0, 0, HB), scalar=0.0,
                in1=chunk(0, 1, HB), op0=ALU.bypass, op1=ALU.mult,
                accum_out=scol(qidx["lt"]))
            vector.wait_ge(csem[1], 16)
            vector.scalar_tensor_tensor(
                out=bcast(3, D // 2), in0=chunk(0, 0, HB), scalar=0.0,
                in1=chunk(1, 0, HB), op0=ALU.bypass, op1=ALU.mult,
                accum_out=scol(qidx["ln"]))
            vector.scalar_tensor_tensor(
                out=bcast(4, D // 2), in0=chunk(0, 1, HB), scalar=0.0,
                in1=chunk(1, 1, HB), op0=ALU.bypass, op1=ALU.mult,
                accum_out=scol(qidx["tn"]))
            # lt_j needs only chunk j: run it one chunk ahead of ln/tn
            vector.scalar_tensor_tensor(
                out=bcast(2), in0=chunk(1, 0), scalar=0.0,
                in1=chunk(1, 1), op0=ALU.bypass, op1=ALU.mult,
                accum_out=rcol("lt", 1))
            for j in range(1, g - 1):
                vector.wait_ge(semof(j + 1), 16)
                if j < g - 2:
                    vector.scalar_tensor_tensor(
                        out=bcast(2), in0=chunk(j + 1, 0), scalar=0.0,
                        in1=chunk(j + 1, 1), op0=ALU.bypass, op1=ALU.mult,
                        accum_out=rcol("lt", j + 1))
                ins = vector.scalar_tensor_tensor(
                    out=bcast(3), in0=chunk(j, 0), scalar=0.0,
                    in1=chunk(j + 1, 0), op0=ALU.bypass, op1=ALU.mult,
                    accum_out=rcol("ln", j))
                if j not in pool_ord:
                    ins = vector.scalar_tensor_tensor(
                        out=bcast(4), in0=chunk(j, 1), scalar=0.0,
                        in1=chunk(j + 1, 1), op0=ALU.bypass, op1=ALU.mult,
                        accum_out=rcol("tn", j))
                if j == g - 2:
                    ins.then_inc(part_sem, 1)
            # last chunk's lt, deferred so part_sem fires as early as possible
            vector.scalar_tensor_tensor(
                out=bcast(2), in0=chunk(g - 1, 0), scalar=0.0,
                in1=chunk(g - 1, 1), op0=ALU.bypass, op1=ALU.mult,
                accum_out=rcol("lt", g - 1)).then_inc(done_sem, 1)

        @block.gpsimd
        def _(gpsimd):
            # the j = g-1 ln/tn columns are host-filled; zero them so the
            # out-b DMA never reads uninitialized SBUF
            gpsimd.memset(res.ap()[0:parts, acols + 3:acols + 5], 0)
            for o, j in enumerate(pool_set):
                gpsimd.wait_ge(semof(j + 1), 16)
                if o >= 3:
                    gpsimd.wait_ge(pcons_sem, o - 2)
                gpsimd.tensor_tensor(
                    out=pslot[0:parts, o % 3, :], in0=chunk(j, 1),
                    in1=chunk(j + 1, 1), op=ALU.mult).then_inc(pmul_sem, 1)

    _cached[key] = nc
    return nc


def _run_device(logits, tgt_out, valid, trace=False):
    """Returns dict q -> (B, S) float32 row-dot arrays (zeros at unused
    positions), plus kernel results."""
    lbg = np.ascontiguousarray(np.swapaxes(logits, 0, 1)).reshape(B * S, D)
    tbg = np.ascontiguousarray(np.swapaxes(tgt_out, 0, 1)).reshape(B * S, D)
    nrows = B * S
    vflat = valid.reshape(-1)
    vp_all = np.flatnonzero(vflat)
    tot = len(vp_all)
    # balanced split of the global valid-row stream; cuts land where the
    # two rows either side are NOT an adjacent valid pair, so no valid
    # pair straddles cores
    cuts = [0]
    for c in range(1, N_CORES):
        t = (tot * c) // N_CORES
        while 0 < t < tot and vp_all[t] == vp_all[t - 1] + 1:
            t += 1
        cuts.append(min(t, tot))
    cuts.append(tot)
    vps = [vp_all[cuts[c]:cuts[c + 1]] for c in range(N_CORES)]
    kmax = max((len(vp) for vp in vps), default=1)
    g = min(16, max(4, -(-kmax // P)))
    parts = min(P, -(-kmax // g))
    rows_c = parts * g
    nc = _build_program(g, parts)
    in_maps = []
    for c in range(N_CORES):
        vp = vps[c]
        x = np.zeros((rows_c, 2, D), np.float32)
        x[:len(vp), 0] = lbg[vp]
        x[:len(vp), 1] = tbg[vp]
        in_maps.append({"x": x})
    kres = run_bass_kernel_spmd(nc, in_maps, list(range(N_CORES)), trace=trace)
    # host-side boundary pairs: compacted rows (g*p+g-1, g*p+g) share no
    # chunk window on-device; their ln/tn come from a tiny host einsum
    hb = {}
    for c in range(N_CORES):
        vp = vps[c]
        r = np.arange(g - 1, rows_c - 1, g)
        r = r[r + 1 < len(vp)]
        a, b = vp[r], vp[r + 1]
        hb[c] = (r,
                 np.einsum('ij,ij->i', lbg[a], lbg[b], dtype=np.float64),
                 np.einsum('ij,ij->i', tbg[a], tbg[b], dtype=np.float64))
    pos = np.cumsum(vflat) - 1                 # orig row -> compacted idx
    rs = np.flatnonzero(vflat[:-1] & vflat[1:])
    full = {}
    for i, q in enumerate(QUANTS):
        cat = np.zeros(tot + 1, np.float64)
        for c in range(N_CORES):
            r = kres.results[c]["res"]
            arr = r[:, :5 * g].reshape(parts, g, 5)[:, :, i].copy()
            arr[:, 0] += r[:, 5 * g + i]       # chunk 0's B-half
            vals = arr.reshape(rows_c).astype(np.float64)
            if q in ("ln", "tn"):
                ridx, lnb, tnb = hb[c]
                vals[ridx] = lnb if q == "ln" else tnb
            cat[cuts[c]:cuts[c + 1]] = vals[:len(vps[c])]
        fullr = np.zeros(nrows, np.float64)
        if q in ("ll", "tt", "lt"):
            fullr[vp_all] = cat[:tot]
        else:
            # pair (r, r+1), both valid: adjacent compacted positions in
            # one core's stream (cuts avoid valid pairs)
            fullr[rs] = cat[pos[rs]]
        full[q] = fullr.reshape(B, S)
    return full, kres


def _finish_host(rows, mask):
    """Host-side float64 finish: reproduce reference semantics exactly."""
    ll = rows["ll"].astype(np.float64)
    tt = rows["tt"].astype(np.float64)
    lt = rows["lt"].astype(np.float64)
    ln = rows["ln"].astype(np.float64)
    tn = rows["tn"].astype(np.float64)

    valid = ~mask                     # (B, S)
    n_valid = float(valid.sum())

    mse = ((ll - 2.0 * lt + tt) * valid).sum() / (n_valid * D)

    na = np.maximum(np.sqrt(ll), 1e-8)
    nb = np.maximum(np.sqrt(tt), 1e-8)
    c = np.where(valid, lt / (na * nb), 0.0)
    loss_cos = ((1.0 - c) * valid).sum() / n_valid

    nl = np.maximum(np.sqrt(ll), 1e-6)
    nt = np.maximum(np.sqrt(tt), 1e-6)
    d_l = ln[:, :S - 1] / (nl[:, :-1] * nl[:, 1:])
    d_t = tn[:, :S - 1] / (nt[:, :-1] * nt[:, 1:])
    pair_valid = valid[:, :-1] & valid[:, 1:]
    cnt = int(pair_valid.sum())
    loss_delta = (np.square(d_l - d_t) * pair_valid).sum() / max(cnt, 1)

    L = B * (S - 1)
    pvf = pair_valid.reshape(-1)

    def dd(d_flat):
        dense = np.zeros(L, np.float64)
        dense[:cnt] = d_flat[pvf]
        prev = dense[:-1]
        den = np.where(prev != 0, prev, 1e-6)
        return (dense[1:] - prev) / den

    dd_l = dd(d_l.reshape(-1))
    dd_t = dd(d_t.reshape(-1))
    dd_valid = np.arange(L - 1) < (cnt - 1)
    n_dd = float(max(cnt - 1, 1))
    loss_dd = (np.square(dd_l - dd_t) * dd_valid).sum() / n_dd / 100.0

    return mse + loss_cos + loss_delta + loss_dd


def kernel(logits, tgt_out, tgt_padding_mask, _trace=False):
    logits = np.asarray(logits, dtype=np.float32)
    tgt_out = np.asarray(tgt_out, dtype=np.float32)
    mask = np.asarray(tgt_padding_mask).astype(bool)
    rows, kres = _run_device(logits, tgt_out, ~mask, trace=_trace)
    total = _finish_host(rows, mask)
    out = np.array(total, dtype=np.float32)
    if _trace:
        return out, kres
    return out


# revision 10
# speedup vs baseline: 1.1222x; 1.0116x over previous
"""Trainium2 Bass kernel for nn_Mixture_Loss_74053826118054.

Strategy (pure data parallel: batch axis B=256 sharded over 8 cores):
  Every term of the loss depends only on 5 per-(s,b)-row reductions over D:
    ll = sum_d l^2,  tt = sum_d t^2,  lt = sum_d l*t,
    ln = sum_d l[s]*l[s+1],  tn = sum_d t[s]*t[s+1]
  The tiny O(S*B) finish (cos, deltas, rank-compaction, delta-of-delta)
  runs on host in float64, reproducing the reference semantics exactly.

Mask compaction: each core receives only its VALID rows, densely packed in
original order. Row r maps to (partition p, chunk j) with r = g*p + j, so
consecutive rows stay adjacent within a partition's g-row window; the
window-boundary pairs (r = g*p + g-1) are computed on the HOST (~127 pairs
per core, microseconds in numpy) so no overlap chunk is ever DMA'd.
DMA is trimmed to parts = ceil(kmax/g) partitions so only real rows move.

Engine assignment (v5, from HW traces):
  - ACT: ll, tt squares (1.04 us per [128,1024] Square+accumulate).
  - DVE: lt (one chunk ahead), ln, tn as fused scalar_tensor_tensor
    product+accumulate (1.14 us/op; no faster f32 path exists: the DVE
    2x/4x perf modes need 2-byte dtypes, and fp16/bf16 inputs fail
    accuracy by 25x/126x because loss_dd amplifies tiny cosine-delta
    errors ~1000x).
  - GpSimd/Pool compute is poison: a Pool tensor op slows concurrent DVE
    ops 2.9x (measured), so Pool only zero-fills the two host-owned
    result columns.
  - Input DMAs are issued in the engine preamble (before the block
    barrier) so the DGE pipeline starts ~2 us earlier; chunks 0/1 are
    split into l/t jobs so the first DVE/ACT ops start on first data.
  DVE busy ~25 us/core is the stream bound (DMA floor is ~22 us:
  16 rings x ~26.7 GB/s for ~8.4 MB/core of f32 rows); the remaining
  ~14 us are NEFF-fixed (walrus-injected 253-semaphore epilogue clear
  ~8.8 us + block entry + first-DMA latency).
"""

import numpy as np

from contextlib import ExitStack

import concourse.bass as bass
import concourse.mybir as mybir
from concourse.bass_utils import run_bass_kernel_spmd

F32 = mybir.dt.float32
AF = mybir.ActivationFunctionType
ALU = mybir.AluOpType

N_CORES = 8
S, B, D = 64, 256, 1024
P = 128                         # max partitions per tile
QUANTS = ("ll", "tt", "lt", "ln", "tn")

_cached = {}


def _build_program(g, parts):
    """g = rows per partition window; parts = partitions carrying rows."""
    key = (g, parts)
    if key in _cached:
        return _cached[key]
    rows_c = parts * g
    nc = bass.Bass()
    ncols = 5 * g
    x_in = nc.dram_tensor("x", [rows_c, 2, D], F32, kind="ExternalInput")
    res_out = nc.dram_tensor("res", [parts, ncols], F32,
                             kind="ExternalOutput")
    x_v = x_in.rearrange("(w g) v d -> w g v d", g=g)

    with ExitStack() as stack:
        ec = stack.enter_context
        c01 = [[ec(nc.semaphore(f"c{j}{v}")) for v in "lt"] for j in (0, 1)]
        csem = {j: ec(nc.semaphore(f"c{j}"))
                for j in range(2, min(g, 14))}
        xbig = ec(nc.sbuf_tensor([P, g * 2 * D], F32))
        dummies = ec(nc.sbuf_tensor([P, 8], F32))
        res = ec(nc.sbuf_tensor([P, ncols], F32))
        part_sem = ec(nc.semaphore("part_sem"))
        done_sem = ec(nc.semaphore("done_sem"))
        out_sem = ec(nc.semaphore("out_sem"))
        qidx = {q: i for i, q in enumerate(QUANTS)}
        xc = xbig.ap().rearrange("p (c v d) -> p c v d", v=2, d=D)

        def chunk(j, half):
            return xc[0:parts, j, half, :]

        def rcol(q, j):
            k = 5 * j + qidx[q]
            return res.ap()[0:parts, k:k + 1]

        def bcast(k, n=D):
            return dummies.ap()[0:parts, k:k + 1].broadcast_to((parts, n))

        def semof(j):
            # chunks 13..g-1 arrive as one DMA job on csem[13] (keeps the
            # total job count <= 19, required for full-rate DVE)
            return csem[min(j, 13)]

        acols = 5 * (g - 1)     # out-a column count (chunks 0..g-2)

        # input DMAs issued in the preamble: the DGE pipeline starts
        # filling SBUF while the engines are still entering the block
        for j in (0, 1):
            for v in (0, 1):
                nc.sync.dma_start(
                    out=xc[0:parts, j, v, :],
                    in_=x_v[0:parts, j, v, :]).then_inc(c01[j][v], 16)
        for j in range(2, min(13, g)):
            nc.sync.dma_start(out=xc[0:parts, j, :, :],
                              in_=x_v[0:parts, j, :, :]).then_inc(csem[j], 16)
        if g > 13:
            nc.sync.dma_start(out=xc[0:parts, 13:g, :, :],
                              in_=x_v[0:parts, 13:g, :, :]).then_inc(
                csem[13], 16)

        block = ec(nc.Block())

        @block.sync
        def _(sync):
            sync.wait_ge(part_sem, 2)
            sync.dma_start(out=res_out[:, 0:acols],
                           in_=res.ap()[0:parts, 0:acols]).then_inc(out_sem, 16)
            sync.wait_ge(done_sem, 2)
            sync.dma_start(out=res_out[:, acols:ncols],
                           in_=res.ap()[0:parts, acols:ncols]).then_inc(
                out_sem, 16)
            sync.wait_ge(out_sem, 32)

        @block.scalar
        def _(scalar):
            scalar.wait_ge(c01[0][0], 16)
            scalar.activation(bcast(0), chunk(0, 0), AF.Square,
                              accum_out=rcol("ll", 0))
            scalar.wait_ge(c01[0][1], 16)
            scalar.activation(bcast(1), chunk(0, 1), AF.Square,
                              accum_out=rcol("tt", 0))
            scalar.wait_ge(c01[1][0], 16)
            scalar.activation(bcast(0), chunk(1, 0), AF.Square,
                              accum_out=rcol("ll", 1))
            scalar.wait_ge(c01[1][1], 16)
            ins = scalar.activation(bcast(1), chunk(1, 1), AF.Square,
                                    accum_out=rcol("tt", 1))
            for j in range(2, g):
                if j < 14:
                    scalar.wait_ge(semof(j), 16)
                scalar.activation(bcast(0), chunk(j, 0), AF.Square,
                                  accum_out=rcol("ll", j))
                ins = scalar.activation(bcast(1), chunk(j, 1), AF.Square,
                                        accum_out=rcol("tt", j))
                if j == g - 2:
                    ins.then_inc(part_sem, 1)
            ins.then_inc(done_sem, 1)

        @block.vector
        def _(vector):
            vector.wait_ge(c01[0][1], 16)
            vector.scalar_tensor_tensor(
                out=bcast(2), in0=chunk(0, 0), scalar=0.0,
                in1=chunk(0, 1), op0=ALU.bypass, op1=ALU.mult,
                accum_out=rcol("lt", 0))
            vector.wait_ge(c01[1][0], 16)
            vector.scalar_tensor_tensor(
                out=bcast(3), in0=chunk(0, 0), scalar=0.0,
                in1=chunk(1, 0), op0=ALU.bypass, op1=ALU.mult,
                accum_out=rcol("ln", 0))
            vector.wait_ge(c01[1][1], 16)
            vector.scalar_tensor_tensor(
                out=bcast(4), in0=chunk(0, 1), scalar=0.0,
                in1=chunk(1, 1), op0=ALU.bypass, op1=ALU.mult,
                accum_out=rcol("tn", 0))
            # lt_j needs only chunk j: run it one chunk ahead of ln/tn
            vector.scalar_tensor_tensor(
                out=bcast(2), in0=chunk(1, 0), scalar=0.0,
                in1=chunk(1, 1), op0=ALU.bypass, op1=ALU.mult,
                accum_out=rcol("lt", 1))
            for j in range(1, g - 1):
                vector.wait_ge(semof(j + 1), 16)
                if j < g - 2:
                    vector.scalar_tensor_tensor(
                        out=bcast(2), in0=chunk(j + 1, 0), scalar=0.0,
                        in1=chunk(j + 1, 1), op0=ALU.bypass, op1=ALU.mult,
                        accum_out=rcol("lt", j + 1))
                vector.scalar_tensor_tensor(
                    out=bcast(3), in0=chunk(j, 0), scalar=0.0,
                    in1=chunk(j + 1, 0), op0=ALU.bypass, op1=ALU.mult,
                    accum_out=rcol("ln", j))
                ins = vector.scalar_tensor_tensor(
                    out=bcast(4), in0=chunk(j, 1), scalar=0.0,
                    in1=chunk(j + 1, 1), op0=ALU.bypass, op1=ALU.mult,
                    accum_out=rcol("tn", j))
                if j == g - 2:
                    ins.then_inc(part_sem, 1)
            # last chunk's lt, deferred so part_sem fires as early as possible
            vector.scalar_tensor_tensor(
                out=bcast(2), in0=chunk(g - 1, 0), scalar=0.0,
                in1=chunk(g - 1, 1), op0=ALU.bypass, op1=ALU.mult,
                accum_out=rcol("lt", g - 1)).then_inc(done_sem, 1)

        @block.gpsimd
        def _(gpsimd):
            # the j = g-1 ln/tn columns are host-filled; zero them so the
            # out-b DMA never reads uninitialized SBUF
            gpsimd.memset(res.ap()[0:parts, acols + 3:acols + 5], 0)

    _cached[key] = nc
    return nc


def _run_device(logits, tgt_out, valid, trace=False):
    """Returns dict q -> (B, S) float32 row-dot arrays (zeros at unused
    positions), plus kernel results."""
    lbg = np.ascontiguousarray(np.swapaxes(logits, 0, 1)).reshape(B * S, D)
    tbg = np.ascontiguousarray(np.swapaxes(tgt_out, 0, 1)).reshape(B * S, D)
    nrows = B * S
    vflat = valid.reshape(-1)
    vp_all = np.flatnonzero(vflat)
    tot = len(vp_all)
    # balanced split of the global valid-row stream; cuts land where the
    # two rows either side are NOT an adjacent valid pair, so no valid
    # pair straddles cores
    cuts = [0]
    for c in range(1, N_CORES):
        t = (tot * c) // N_CORES
        while 0 < t < tot and vp_all[t] == vp_all[t - 1] + 1:
            t += 1
        cuts.append(min(t, tot))
    cuts.append(tot)
    vps = [vp_all[cuts[c]:cuts[c + 1]] for c in range(N_CORES)]
    kmax = max((len(vp) for vp in vps), default=1)
    g = min(16, max(4, -(-kmax // P)))
    parts = min(P, -(-kmax // g))
    rows_c = parts * g
    nc = _build_program(g, parts)
    in_maps = []
    for c in range(N_CORES):
        vp = vps[c]
        x = np.zeros((rows_c, 2, D), np.float32)
        x[:len(vp), 0] = lbg[vp]
        x[:len(vp), 1] = tbg[vp]
        in_maps.append({"x": x})
    kres = run_bass_kernel_spmd(nc, in_maps, list(range(N_CORES)), trace=trace)
    # host-side boundary pairs: compacted rows (g*p+g-1, g*p+g) share no
    # chunk window on-device; their ln/tn come from a tiny host einsum
    hb = {}
    for c in range(N_CORES):
        vp = vps[c]
        r = np.arange(g - 1, rows_c - 1, g)
        r = r[r + 1 < len(vp)]
        a, b = vp[r], vp[r + 1]
        hb[c] = (r,
                 np.einsum('ij,ij->i', lbg[a], lbg[b], dtype=np.float64),
                 np.einsum('ij,ij->i', tbg[a], tbg[b], dtype=np.float64))
    pos = np.cumsum(vflat) - 1                 # orig row -> compacted idx
    rs = np.flatnonzero(vflat[:-1] & vflat[1:])
    full = {}
    for i, q in enumerate(QUANTS):
        cat = np.zeros(tot + 1, np.float64)
        for c in range(N_CORES):
            r = kres.results[c]["res"]
            arr = r[:, :5 * g].reshape(parts, g, 5)[:, :, i]
            vals = arr.reshape(rows_c).astype(np.float64)
            if q in ("ln", "tn"):
                ridx, lnb, tnb = hb[c]
                vals[ridx] = lnb if q == "ln" else tnb
            cat[cuts[c]:cuts[c + 1]] = vals[:len(vps[c])]
        fullr = np.zeros(nrows, np.float64)
        if q in ("ll", "tt", "lt"):
            fullr[vp_all] = cat[:tot]
        else:
            # pair (r, r+1), both valid: adjacent compacted positions in
            # one core's stream (cuts avoid valid pairs)
            fullr[rs] = cat[pos[rs]]
        full[q] = fullr.reshape(B, S)
    return full, kres


def _finish_host(rows, mask):
    """Host-side float64 finish: reproduce reference semantics exactly."""
    ll = rows["ll"].astype(np.float64)
    tt = rows["tt"].astype(np.float64)
    lt = rows["lt"].astype(np.float64)
    ln = rows["ln"].astype(np.float64)
    tn = rows["tn"].astype(np.float64)

    valid = ~mask                     # (B, S)
    n_valid = float(valid.sum())

    mse = ((ll - 2.0 * lt + tt) * valid).sum() / (n_valid * D)

    na = np.maximum(np.sqrt(ll), 1e-8)
    nb = np.maximum(np.sqrt(tt), 1e-8)
    c = np.where(valid, lt / (na * nb), 0.0)
    loss_cos = ((1.0 - c) * valid).sum() / n_valid

    nl = np.maximum(np.sqrt(ll), 1e-6)
    nt = np.maximum(np.sqrt(tt), 1e-6)
    d_l = ln[:, :S - 1] / (nl[:, :-1] * nl[:, 1:])
    d_t = tn[:, :S - 1] / (nt[:, :-1] * nt[:, 1:])
    pair_valid = valid[:, :-1] & valid[:, 1:]
    cnt = int(pair_valid.sum())
    loss_delta = (np.square(d_l - d_t) * pair_valid).sum() / max(cnt, 1)

    L = B * (S - 1)
    pvf = pair_valid.reshape(-1)

    def dd(d_flat):
        dense = np.zeros(L, np.float64)
        dense[:cnt] = d_flat[pvf]
        prev = dense[:-1]
        den = np.where(prev != 0, prev, 1e-6)
        return (dense[1:] - prev) / den

    dd_l = dd(d_l.reshape(-1))
    dd_t = dd(d_t.reshape(-1))
    dd_valid = np.arange(L - 1) < (cnt - 1)
    n_dd = float(max(cnt - 1, 1))
    loss_dd = (np.square(dd_l - dd_t) * dd_valid).sum() / n_dd / 100.0

    return mse + loss_cos + loss_delta + loss_dd


def kernel(logits, tgt_out, tgt_padding_mask, _trace=False):
    logits = np.asarray(logits, dtype=np.float32)
    tgt_out = np.asarray(tgt_out, dtype=np.float32)
    mask = np.asarray(tgt_padding_mask).astype(bool)
    rows, kres = _run_device(logits, tgt_out, ~mask, trace=_trace)
    total = _finish_host(rows, mask)
    out = np.array(total, dtype=np.float32)
    if _trace:
        return out, kres
    return out
